# revision 16
# baseline (speedup 1.0000x reference)
"""Conv2d 3x3 (N=32, C_in=128, H=W=56, C_out=256, stride 1, pad 1) on 8 TRN2
NeuronCores.

Strategy: data-parallel over batch (4 images per core). Per core the conv is
an implicit-GEMM: C_in=128 is exactly the SBUF partition dim, so each of the
9 filter taps is one 128x128 (C_in x C_out-chunk) stationary matmul over a
shifted spatial window of the zero-padded image held in SBUF. The 9 taps
accumulate in PSUM; bias is fused into the PSUM->SBUF drain on the scalar
engine. Matmuls run in float32r (fp32 with 11-bit mantissa, full PE rate at
free-dim >= 256).

Steady-state pipeline (the timed repeat loop): two persistent sets of padded
image buffers (A/B). Each For_i iteration runs two conv passes: {load B,
compute A} then {load A, compute B}, so the PE never waits for a load at the
loop back-edge -- the data it needs next was loaded a full pass (~50us)
earlier. Loads stage through a dense SBUF tile and one DVE round-copy into
the padded buffer (the BIR verifier requires f32r matmul inputs to come from
a rounding instruction); the zero borders are written once in the prologue
and never touched again. This removes the old per-iteration boundary stall
(~7.4us PE idle + p-state re-ramp): steady state sims at 95.3us/conv vs the
94.1us f32r matmul floor (504 matmuls x 448 free rows @ 2.4GHz).

An fp8e4 DoubleRow variant (_build_fp8, K_FP8=1) computes the same conv via
a hi+lo error-compensated fp8 split (rel err 1.2e-3) and sims at 82.5us, but
measures ~1.9x SLOWER than f32r on real hardware -- DoubleRow does not hit
the cost model's 0.5 cycles/row there. Kept for reference, default off.
"""

import numpy as np

N, C_IN, H, W = 32, 128, 56, 56
C_OUT, KH, KW = 256, 3, 3
NCORES = 8
NIMG = N // NCORES          # images per core
P = 128                     # partitions = C_IN
NCHUNK = C_OUT // P         # C_out chunks of 128
KHW = KH * KW
HP, WP = H + 2, W + 2       # padded image
HT = 8                      # output rows per PSUM tile
NT = H // HT                # 7 h-tiles
FREE = HT * W               # 448 <= 512 fp32 PSUM bank

# fp8 path: matmul free dim runs over CONTIGUOUS padded rows (58 wide, the
# 2 pad columns produce garbage that is never drained), so the DoubleRow
# moving AP is a clean 3D [128, 2, 464]
FREE8 = HT * WP             # 464 <= 512 fp32 PSUM bank
PADN = HP * WP + 2          # +2: last tile's garbage columns read past the end
BASES = [kh * WP + kw for kh in range(KH) for kw in range(KW)]
WSCALE = 256.0              # weights are subnormal in fp8 without this
NPAIR = 14                  # 13 DoubleRow pairs + 1 plain single

_CACHE = {}


def _build(repeat: int = 1, unroll: bool = False):
    import os

    import concourse.tile as tile
    from concourse import bacc, mybir

    f32 = mybir.dt.float32
    f32r = mybir.dt.float32r
    bf16 = mybir.dt.bfloat16

    nc = bacc.Bacc("TRN2", target_bir_lowering=False, debug=False)

    x_d = nc.dram_tensor("x", [NIMG, P, H, W], f32, kind="ExternalInput").ap()
    w_d = nc.dram_tensor("w", [P, KHW, NCHUNK, P], f32, kind="ExternalInput").ap()
    b_d = nc.dram_tensor("b", [P, NCHUNK], f32, kind="ExternalInput").ap()
    out_d = nc.dram_tensor(
        "out", [NIMG, NCHUNK, P, NT, FREE], f32, kind="ExternalOutput"
    ).ap()

    NWU = 20  # PE warmup matmuls (ramp the clock-gate during the load phase)

    cp_eng = os.environ.get("K_CP_ENG", "dve")  # dve | pool

    with tile.TileContext(nc) as tc:
        with (
            tc.tile_pool(name="wpool", bufs=1) as wpool,
            tc.tile_pool(name="xppool", bufs=1) as xppool,
            tc.tile_pool(name="xqpool", bufs=2) as xqpool,
            tc.tile_pool(name="pspool", bufs=7, space="PSUM") as pspool,
            tc.tile_pool(name="obpool", bufs=3) as obpool,
        ):
            # PE warmup: dummy bf16 matmuls that depend only on one tiny
            # memset, so the PE clock-gate is at full rate when the first
            # real matmul's inputs land.
            wu = wpool.tile([P, 256], bf16, tag="wu")
            nc.vector.memset(wu[:], 0.5)
            pswu = pspool.tile([P, 256], f32, tag="pswu", bufs=1)
            for _ in range(NWU):
                nc.tensor.matmul(pswu[:], wu[:, 0:P], wu[:], start=True, stop=True)

            # weights + bias (prologue only; chunk 0 first -- it gates the
            # first compute plane)
            wf = wpool.tile([P, KHW, NCHUNK, P], f32, tag="wf")
            wr = wpool.tile([P, KHW, NCHUNK, P], f32r, tag="wr")
            nc.scalar.dma_start(wf[:, :, 0, :], w_d[:, :, 0, :])
            nc.vector.tensor_copy(wr[:, :, 0, :], wf[:, :, 0, :])
            nc.scalar.dma_start(wf[:, :, 1, :], w_d[:, :, 1, :])
            nc.vector.tensor_copy(wr[:, :, 1, :], wf[:, :, 1, :])
            bt = wpool.tile([P, NCHUNK], f32, tag="bt")
            nc.scalar.dma_start(bt[:], b_d[:])

            zz = wpool.tile([P, 2 * WP], f32, tag="zz")
            nc.vector.memset(zz[:], 0.0)

            # two persistent sets of padded image buffers; borders zeroed
            # once here (loads only ever write the interior)
            xps = []
            for s in range(2):
                row = []
                for i in range(NIMG):
                    xp = xppool.tile([P, HP * WP], f32r, tag=f"xp{s}{i}")
                    xp3 = xp[:].rearrange("p (h w) -> p h w", w=WP)
                    nc.vector.tensor_copy(xp[:, 0:WP], zz[:, 0:WP])
                    nc.vector.tensor_copy(
                        xp[:, (HP - 1) * WP : HP * WP], zz[:, 0:WP]
                    )
                    side = xp[:, WP - 1 : WP - 1 + (HP - 1) * WP].rearrange(
                        "p (a b) -> p a b", b=WP
                    )[:, :, 0:2]
                    nc.vector.tensor_copy(side, zz[:, 0 : 2 * (HP - 1)])
                    row.append(xp3)
                xps.append(row)

            # the BIR verifier requires every producer of an f32r matmul
            # input to be a rounding instruction, so loads stage through a
            # dense SBUF tile and a round-copy into the padded buffer
            cp = nc.pool.tensor_copy if cp_eng == "pool" else nc.vector.tensor_copy

            def emit_loads(dst):
                for i in range(NIMG):
                    xq = xqpool.tile([P, H, W], f32, tag="xq", name=f"xq{dst}_{i}")
                    nc.sync.dma_start(xq[:], x_d[i])
                    cp(xps[dst][i][:, 1 : 1 + H, 1 : 1 + W], xq[:])

            def emit_plane(img, c, xp3, tail=False):
                ob = obpool.tile([P, NT, FREE], f32, tag="ob", name=f"ob{img}_{c}")
                for t in range(NT):
                    ps = pspool.tile(
                        [P, FREE], f32, tag="ps", name=f"ps{img}_{c}_{t}"
                    )
                    for k in range(KHW):
                        kh, kw = divmod(k, KW)
                        rhs = xp3[:, t * HT + kh : t * HT + kh + HT, kw : kw + W]
                        nc.tensor.matmul(
                            ps[:], wr[:, k, c, :], rhs,
                            start=(k == 0), stop=(k == KHW - 1),
                        )
                    nc.scalar.activation(
                        ob[:, t, :], ps[:],
                        mybir.ActivationFunctionType.Identity,
                        bias=bt[:, c : c + 1],
                    )
                    # stage the plane, store as two half-plane DMAs -- except
                    # the very last plane, which streams out tile-by-tile to
                    # keep the kernel tail short
                    if tail:
                        nc.scalar.dma_start(out_d[img, c, :, t, :], ob[:, t, :])
                    elif t == 2:
                        nc.scalar.dma_start(out_d[img, c, :, 0:3, :], ob[:, 0:3, :])
                if not tail:
                    nc.scalar.dma_start(out_d[img, c, :, 3:NT, :], ob[:, 3:NT, :])

            def emit_pass(src, dst=None, last=False):
                if dst is not None:
                    emit_loads(dst)
                for c in range(NCHUNK):
                    for img in range(NIMG):
                        emit_plane(
                            img, c, xps[src][img],
                            tail=last and c == NCHUNK - 1 and img == NIMG - 1,
                        )

            emit_loads(0)  # prologue: fill set A

            if repeat == 1:
                emit_pass(0, None, last=True)
            else:
                if unroll:
                    for _ in range(repeat // 2):
                        emit_pass(0, 1)
                        emit_pass(1, 0)
                else:
                    with tc.For_i(
                        0, repeat // 2, 1,
                        staggered_reset=True,
                        hint_engines=(
                            mybir.EngineType.PE,
                            mybir.EngineType.SP,
                            mybir.EngineType.Activation,
                            mybir.EngineType.DVE,
                        ),
                    ):
                        emit_pass(0, 1)
                        emit_pass(1, 0)
                if repeat % 2:
                    # odd repeat: one final pass outside the loop (set A is
                    # freshly loaded by the last in-loop pass)
                    emit_pass(0, None, last=True)

    nc.compile()
    return nc


def _build_fp8(repeat: int = 1, unroll: bool = False):
    """fp8e4 DoubleRow variant: out ~= (whi+wlo)*xhi + whi*xlo, with
    xhi = fp8(x), xlo = fp8(x - xhi), w* = fp8 split of w*WSCALE (split on
    host). 27 tap-passes pack into 13 K=256 DoubleRow matmuls (0.5 cyc/row)
    + 1 plain fp8 matmul per PSUM tile -> 3480 PE cycles/tile vs f32r 4032.
    The 1/WSCALE unscale folds into the activation drain."""
    import os

    import concourse.tile as tile
    from concourse import bacc, mybir
    from concourse.ap import AP

    f32 = mybir.dt.float32
    fp8 = mybir.dt.float8e4
    bf16 = mybir.dt.bfloat16
    DR = mybir.MatmulPerfMode.DoubleRow

    nc = bacc.Bacc("TRN2", target_bir_lowering=False, debug=False)

    x_d = nc.dram_tensor("x", [NIMG, P, H, W], f32, kind="ExternalInput").ap()
    w_d = nc.dram_tensor(
        "w8", [P, NPAIR, 2, NCHUNK, P], fp8, kind="ExternalInput"
    ).ap()
    b_d = nc.dram_tensor("b", [P, NCHUNK], f32, kind="ExternalInput").ap()
    out_d = nc.dram_tensor(
        "out", [NIMG, NCHUNK, P, NT, FREE], f32, kind="ExternalOutput"
    ).ap()

    NWU = 20

    with tile.TileContext(nc) as tc:
        with (
            tc.tile_pool(name="wpool", bufs=1) as wpool,
            tc.tile_pool(name="xppool", bufs=1) as xppool,
            tc.tile_pool(name="xqpool", bufs=2) as xqpool,
            tc.tile_pool(name="pspool", bufs=7, space="PSUM") as pspool,
            tc.tile_pool(name="obpool", bufs=3) as obpool,
        ):
            wu = wpool.tile([P, 256], bf16, tag="wu")
            nc.vector.memset(wu[:], 0.5)
            pswu = pspool.tile([P, 256], f32, tag="pswu", bufs=1)
            for _ in range(NWU):
                nc.tensor.matmul(pswu[:], wu[:, 0:P], wu[:], start=True, stop=True)

            # weights DMA directly as fp8 (host-prepared pair layout)
            wt = wpool.tile([P, NPAIR, 2, NCHUNK, P], fp8, tag="wt")
            nc.scalar.dma_start(wt[:], w_d)
            bt = wpool.tile([P, NCHUNK], f32, tag="bt")
            nc.scalar.dma_start(bt[:], b_d)

            # persistent padded fp8 image buffers: [set][img] -> tile
            # [P, 2, PADN] (dim1: 0=xhi, 1=xlo); zeroed once -- loads only
            # ever write the 56x56 interior of each 58x58 plane
            xts = []
            for s in range(2):
                row = []
                for i in range(NIMG):
                    xt = xppool.tile([P, 2, PADN], fp8, tag=f"xt{s}{i}")
                    nc.vector.memset(xt[:], 0.0)
                    row.append(xt)
                xts.append(row)

            def interior(xt, half):
                return xt[:, half, 0 : HP * WP].rearrange(
                    "p (h w) -> p h w", w=WP
                )[:, 1 : 1 + H, 1 : 1 + W]

            def emit_loads(dst):
                for i in range(NIMG):
                    xq = xqpool.tile([P, H, W], f32, tag="xq", name=f"xq{dst}_{i}")
                    nc.sync.dma_start(xq[:], x_d[i])
                    xt = xts[dst][i]
                    nc.vector.tensor_copy(interior(xt, 0), xq[:])
                    nc.vector.tensor_sub(interior(xt, 1), xq[:], interior(xt, 0))

            def emit_plane(img, c, xt, tail=False):
                ob = obpool.tile(
                    [P, NT, HT, W], f32, tag="ob", name=f"ob{img}_{c}"
                )
                pstride = 2 * PADN
                xh = xt[:].tensor
                for t in range(NT):
                    ps = pspool.tile(
                        [P, FREE8], f32, tag="ps", name=f"ps{img}_{c}_{t}"
                    )
                    # 9 DoubleRow pairs (whi[k]: xhi then xlo; dim1 hops the
                    # hi->lo section), 4 DoubleRow pairs (wlo[2m], wlo[2m+1]:
                    # both xhi; dim1 hops between tap bases), 1 plain single
                    # (wlo[8] * xhi)
                    for j in range(9):
                        rhs = AP(
                            xh, t * FREE8 + BASES[j],
                            [[pstride, P], [PADN, 2], [1, FREE8]],
                        )
                        nc.tensor.matmul(
                            ps[:], wt[:, j, :, c, :], rhs,
                            start=(j == 0), stop=False, perf_mode=DR,
                        )
                    for m in range(4):
                        rhs = AP(
                            xh, t * FREE8 + BASES[2 * m],
                            [[pstride, P],
                             [BASES[2 * m + 1] - BASES[2 * m], 2],
                             [1, FREE8]],
                        )
                        nc.tensor.matmul(
                            ps[:], wt[:, 9 + m, :, c, :], rhs,
                            start=False, stop=False, perf_mode=DR,
                        )
                    rhs = AP(
                        xh, t * FREE8 + BASES[8],
                        [[pstride, P], [1, FREE8]],
                    )
                    nc.tensor.matmul(
                        ps[:], wt[:, 13, 0, c, :], rhs, start=False, stop=True
                    )
                    ps3 = ps[:].rearrange("p (r q) -> p r q", q=WP)[:, :, 0:W]
                    nc.scalar.activation(
                        ob[:, t], ps3,
                        mybir.ActivationFunctionType.Identity,
                        bias=bt[:, c : c + 1], scale=1.0 / WSCALE,
                    )
                    if tail:
                        nc.scalar.dma_start(out_d[img, c, :, t, :], ob[:, t])
                    elif t == 2:
                        nc.scalar.dma_start(out_d[img, c, :, 0:3, :], ob[:, 0:3])
                if not tail:
                    nc.scalar.dma_start(out_d[img, c, :, 3:NT, :], ob[:, 3:NT])

            def emit_pass(src, dst=None, last=False):
                if dst is not None:
                    emit_loads(dst)
                for c in range(NCHUNK):
                    for img in range(NIMG):
                        emit_plane(
                            img, c, xts[src][img],
                            tail=last and c == NCHUNK - 1 and img == NIMG - 1,
                        )

            emit_loads(0)

            if repeat == 1:
                emit_pass(0, None, last=True)
            else:
                if unroll:
                    for _ in range(repeat // 2):
                        emit_pass(0, 1)
                        emit_pass(1, 0)
                else:
                    with tc.For_i(
                        0, repeat // 2, 1,
                        staggered_reset=True,
                        hint_engines=(
                            mybir.EngineType.PE,
                            mybir.EngineType.SP,
                            mybir.EngineType.Activation,
                            mybir.EngineType.DVE,
                        ),
                    ):
                        emit_pass(0, 1)
                        emit_pass(1, 0)
                if repeat % 2:
                    emit_pass(0, None, last=True)

    nc.compile()
    return nc


def _use_fp8():
    import os

    return os.environ.get("K_FP8", "0") == "1"


def kernel(x: np.ndarray, weight: np.ndarray, bias: np.ndarray) -> np.ndarray:
    from concourse.bass_utils import run_bass_kernel_spmd

    if "nc" not in _CACHE:
        _CACHE["nc"] = _build_fp8() if _use_fp8() else _build()
    nc = _CACHE["nc"]

    in_maps = [m for m in make_in_maps(x, weight, bias)]
    res = run_bass_kernel_spmd(nc, in_maps, list(range(NCORES)))
    out = np.concatenate(
        [r["out"].reshape(NIMG, C_OUT, H, W) for r in res.results], axis=0
    )
    return out


def make_in_maps(x, weight, bias):
    x = np.ascontiguousarray(x, dtype=np.float32)
    # w layout: [ci, kh*KW+kw, c, co_within_chunk]
    w_t = np.ascontiguousarray(
        weight.astype(np.float32)
        .transpose(1, 2, 3, 0)
        .reshape(P, KHW, NCHUNK, P)
    )
    b_t = np.ascontiguousarray(bias.astype(np.float32).reshape(NCHUNK, P).T)
    base = {"x": None, "w": w_t, "b": b_t}
    if _use_fp8():
        import ml_dtypes

        f8 = ml_dtypes.float8_e4m3
        wS = w_t * WSCALE
        w_hi = wS.astype(f8)
        w_lo = (wS - w_hi.astype(np.float32)).astype(f8)
        wp8 = np.zeros((P, NPAIR, 2, NCHUNK, P), f8)
        for j in range(KHW):           # pairs 0-8: whi[j] in both slots
            wp8[:, j, 0] = w_hi[:, j]
            wp8[:, j, 1] = w_hi[:, j]
        for m in range(4):             # pairs 9-12: (wlo[2m], wlo[2m+1])
            wp8[:, 9 + m, 0] = w_lo[:, 2 * m]
            wp8[:, 9 + m, 1] = w_lo[:, 2 * m + 1]
        wp8[:, 13, 0] = w_lo[:, 8]     # single
        base["w8"] = wp8
    return [
        {**base, "x": x[i * NIMG : (i + 1) * NIMG]} for i in range(NCORES)
    ]


# revision 17
# speedup vs baseline: 1.0417x; 1.0417x over previous
"""Conv2d 3x3 (N=32, C_in=128, H=W=56, C_out=256, stride 1, pad 1) on 8 TRN2
NeuronCores.

Strategy: data-parallel over batch (4 images per core). Per core the conv is
an implicit-GEMM: C_in=128 is exactly the SBUF partition dim, so each of the
9 filter taps is one 128x128 (C_in x C_out-chunk) stationary matmul over a
shifted spatial window of the zero-padded image held in SBUF. The 9 taps
accumulate in PSUM; bias is fused into the PSUM->SBUF drain on the scalar
engine. Matmuls run in float32r (fp32 with 11-bit mantissa, full PE rate at
free-dim >= 256).

Steady-state pipeline (the timed repeat loop): two persistent sets of padded
image buffers (A/B). Each For_i iteration runs two conv passes: {load B,
compute A} then {load A, compute B}, so the PE never waits for a load at the
loop back-edge -- the data it needs next was loaded a full pass (~50us)
earlier. Loads stage through a dense SBUF tile and one DVE round-copy into
the padded buffer (the BIR verifier requires f32r matmul inputs to come from
a rounding instruction); the zero borders are written once in the prologue
and never touched again. This removes the old per-iteration boundary stall
(~7.4us PE idle + p-state re-ramp): steady state sims at 95.3us/conv vs the
94.1us f32r matmul floor (504 matmuls x 448 free rows @ 2.4GHz).

An fp8e4 DoubleRow variant (_build_fp8, K_FP8=1) computes the same conv via
a hi+lo error-compensated fp8 split (rel err 1.2e-3) and sims at 82.5us, but
measures ~1.9x SLOWER than f32r on real hardware -- DoubleRow does not hit
the cost model's 0.5 cycles/row there. Kept for reference, default off.
"""

import numpy as np

N, C_IN, H, W = 32, 128, 56, 56
C_OUT, KH, KW = 256, 3, 3
NCORES = 8
NIMG = N // NCORES          # images per core
P = 128                     # partitions = C_IN
NCHUNK = C_OUT // P         # C_out chunks of 128
KHW = KH * KW
HP, WP = H + 2, W + 2       # padded image
HT = 8                      # output rows per PSUM tile
NT = H // HT                # 7 h-tiles
FREE = HT * W               # 448 <= 512 fp32 PSUM bank

# fp8 path: matmul free dim runs over CONTIGUOUS padded rows (58 wide, the
# 2 pad columns produce garbage that is never drained), so the DoubleRow
# moving AP is a clean 3D [128, 2, 464]
FREE8 = HT * WP             # 464 <= 512 fp32 PSUM bank
PADN = HP * WP + 2          # +2: last tile's garbage columns read past the end
BASES = [kh * WP + kw for kh in range(KH) for kw in range(KW)]
WSCALE = 256.0              # weights are subnormal in fp8 without this
NPAIR = 14                  # 13 DoubleRow pairs + 1 plain single

_CACHE = {}


def _build(repeat: int = 1, unroll: bool = False):
    import os

    import concourse.tile as tile
    from concourse import bacc, mybir

    f32 = mybir.dt.float32
    f32r = mybir.dt.float32r
    bf16 = mybir.dt.bfloat16

    nc = bacc.Bacc("TRN2", target_bir_lowering=False, debug=False)

    x_d = nc.dram_tensor("x", [NIMG, P, H, W], f32, kind="ExternalInput").ap()
    w_d = nc.dram_tensor("w", [P, KHW, NCHUNK, P], f32, kind="ExternalInput").ap()
    b_d = nc.dram_tensor("b", [P, NCHUNK], f32, kind="ExternalInput").ap()
    out_d = nc.dram_tensor(
        "out", [NIMG, NCHUNK, P, NT, FREE], f32, kind="ExternalOutput"
    ).ap()

    NWU = 20  # PE warmup matmuls (ramp the clock-gate during the load phase)

    cp_eng = os.environ.get("K_CP_ENG", "dve")  # dve | pool
    mm_dt_name = os.environ.get("K_MM_DTYPE", "f32r")  # f32r | bf16

    with tile.TileContext(nc) as tc:
        with (
            tc.tile_pool(name="wpool", bufs=1) as wpool,
            tc.tile_pool(name="xppool", bufs=1) as xppool,
            tc.tile_pool(name="xqpool", bufs=2) as xqpool,
            tc.tile_pool(name="pspool", bufs=7, space="PSUM") as pspool,
            tc.tile_pool(name="obpool", bufs=3) as obpool,
        ):
            # PE warmup: dummy bf16 matmuls that depend only on one tiny
            # memset, so the PE clock-gate is at full rate when the first
            # real matmul's inputs land.
            wu = wpool.tile([P, 256], bf16, tag="wu")
            nc.vector.memset(wu[:], 0.5)
            pswu = pspool.tile([P, 256], f32, tag="pswu", bufs=1)
            for _ in range(NWU):
                nc.tensor.matmul(pswu[:], wu[:, 0:P], wu[:], start=True, stop=True)

            # weights + bias (prologue only; chunk 0 first -- it gates the
            # first compute plane)
            mm_dt = {"f32r": f32r, "bf16": bf16}[mm_dt_name]
            wf = wpool.tile([P, KHW, NCHUNK, P], f32, tag="wf")
            wr = wpool.tile([P, KHW, NCHUNK, P], mm_dt, tag="wr")
            nc.scalar.dma_start(wf[:, :, 0, :], w_d[:, :, 0, :])
            nc.vector.tensor_copy(wr[:, :, 0, :], wf[:, :, 0, :])
            nc.scalar.dma_start(wf[:, :, 1, :], w_d[:, :, 1, :])
            nc.vector.tensor_copy(wr[:, :, 1, :], wf[:, :, 1, :])
            bt = wpool.tile([P, NCHUNK], f32, tag="bt")
            nc.scalar.dma_start(bt[:], b_d[:])

            zz = wpool.tile([P, 2 * WP], f32, tag="zz")
            nc.vector.memset(zz[:], 0.0)

            # two persistent sets of padded image buffers; borders zeroed
            # once here (loads only ever write the interior)
            xps = []
            for s in range(2):
                row = []
                for i in range(NIMG):
                    xp = xppool.tile([P, HP * WP], mm_dt, tag=f"xp{s}{i}")
                    xp3 = xp[:].rearrange("p (h w) -> p h w", w=WP)
                    nc.vector.tensor_copy(xp[:, 0:WP], zz[:, 0:WP])
                    nc.vector.tensor_copy(
                        xp[:, (HP - 1) * WP : HP * WP], zz[:, 0:WP]
                    )
                    side = xp[:, WP - 1 : WP - 1 + (HP - 1) * WP].rearrange(
                        "p (a b) -> p a b", b=WP
                    )[:, :, 0:2]
                    nc.vector.tensor_copy(side, zz[:, 0 : 2 * (HP - 1)])
                    row.append(xp3)
                xps.append(row)

            # the BIR verifier requires every producer of an f32r matmul
            # input to be a rounding instruction, so loads stage through a
            # dense SBUF tile and a round-copy into the padded buffer
            cp = nc.pool.tensor_copy if cp_eng == "pool" else nc.vector.tensor_copy

            def emit_loads(dst):
                for i in range(NIMG):
                    xq = xqpool.tile([P, H, W], f32, tag="xq", name=f"xq{dst}_{i}")
                    nc.sync.dma_start(xq[:], x_d[i])
                    cp(xps[dst][i][:, 1 : 1 + H, 1 : 1 + W], xq[:])

            def emit_plane(img, c, xp3, tail=False):
                ob = obpool.tile([P, NT, FREE], f32, tag="ob", name=f"ob{img}_{c}")
                for t in range(NT):
                    ps = pspool.tile(
                        [P, FREE], f32, tag="ps", name=f"ps{img}_{c}_{t}"
                    )
                    for k in range(KHW):
                        kh, kw = divmod(k, KW)
                        rhs = xp3[:, t * HT + kh : t * HT + kh + HT, kw : kw + W]
                        nc.tensor.matmul(
                            ps[:], wr[:, k, c, :], rhs,
                            start=(k == 0), stop=(k == KHW - 1),
                        )
                    nc.scalar.activation(
                        ob[:, t, :], ps[:],
                        mybir.ActivationFunctionType.Identity,
                        bias=bt[:, c : c + 1],
                    )
                    # stage the plane, store as two half-plane DMAs -- except
                    # the very last plane, which streams out tile-by-tile to
                    # keep the kernel tail short
                    if tail:
                        nc.scalar.dma_start(out_d[img, c, :, t, :], ob[:, t, :])
                    elif t == 2:
                        nc.scalar.dma_start(out_d[img, c, :, 0:3, :], ob[:, 0:3, :])
                if not tail:
                    nc.scalar.dma_start(out_d[img, c, :, 3:NT, :], ob[:, 3:NT, :])

            def emit_pass(src, dst=None, last=False):
                if dst is not None:
                    emit_loads(dst)
                for c in range(NCHUNK):
                    for img in range(NIMG):
                        emit_plane(
                            img, c, xps[src][img],
                            tail=last and c == NCHUNK - 1 and img == NIMG - 1,
                        )

            emit_loads(0)  # prologue: fill set A

            if repeat == 1:
                emit_pass(0, None, last=True)
            else:
                if unroll:
                    for _ in range(repeat // 2):
                        emit_pass(0, 1)
                        emit_pass(1, 0)
                else:
                    with tc.For_i(
                        0, repeat // 2, 1,
                        staggered_reset=True,
                        hint_engines=(
                            mybir.EngineType.PE,
                            mybir.EngineType.SP,
                            mybir.EngineType.Activation,
                            mybir.EngineType.DVE,
                        ),
                    ):
                        emit_pass(0, 1)
                        emit_pass(1, 0)
                if repeat % 2:
                    # odd repeat: one final pass outside the loop (set A is
                    # freshly loaded by the last in-loop pass)
                    emit_pass(0, None, last=True)

    nc.compile()
    return nc


def _build_fp8(repeat: int = 1, unroll: bool = False):
    """fp8e4 DoubleRow variant: out ~= (whi+wlo)*xhi + whi*xlo, with
    xhi = fp8(x), xlo = fp8(x - xhi), w* = fp8 split of w*WSCALE (split on
    host). 27 tap-passes pack into 13 K=256 DoubleRow matmuls (0.5 cyc/row)
    + 1 plain fp8 matmul per PSUM tile -> 3480 PE cycles/tile vs f32r 4032.
    The 1/WSCALE unscale folds into the activation drain."""
    import os

    import concourse.tile as tile
    from concourse import bacc, mybir
    from concourse.ap import AP

    f32 = mybir.dt.float32
    fp8 = mybir.dt.float8e4
    bf16 = mybir.dt.bfloat16
    DR = mybir.MatmulPerfMode.DoubleRow

    nc = bacc.Bacc("TRN2", target_bir_lowering=False, debug=False)

    x_d = nc.dram_tensor("x", [NIMG, P, H, W], f32, kind="ExternalInput").ap()
    w_d = nc.dram_tensor(
        "w8", [P, NPAIR, 2, NCHUNK, P], fp8, kind="ExternalInput"
    ).ap()
    b_d = nc.dram_tensor("b", [P, NCHUNK], f32, kind="ExternalInput").ap()
    out_d = nc.dram_tensor(
        "out", [NIMG, NCHUNK, P, NT, FREE], f32, kind="ExternalOutput"
    ).ap()

    NWU = 20

    with tile.TileContext(nc) as tc:
        with (
            tc.tile_pool(name="wpool", bufs=1) as wpool,
            tc.tile_pool(name="xppool", bufs=1) as xppool,
            tc.tile_pool(name="xqpool", bufs=2) as xqpool,
            tc.tile_pool(name="pspool", bufs=7, space="PSUM") as pspool,
            tc.tile_pool(name="obpool", bufs=3) as obpool,
        ):
            wu = wpool.tile([P, 256], bf16, tag="wu")
            nc.vector.memset(wu[:], 0.5)
            pswu = pspool.tile([P, 256], f32, tag="pswu", bufs=1)
            for _ in range(NWU):
                nc.tensor.matmul(pswu[:], wu[:, 0:P], wu[:], start=True, stop=True)

            # weights DMA directly as fp8 (host-prepared pair layout)
            wt = wpool.tile([P, NPAIR, 2, NCHUNK, P], fp8, tag="wt")
            nc.scalar.dma_start(wt[:], w_d)
            bt = wpool.tile([P, NCHUNK], f32, tag="bt")
            nc.scalar.dma_start(bt[:], b_d)

            # persistent padded fp8 image buffers: [set][img] -> tile
            # [P, 2, PADN] (dim1: 0=xhi, 1=xlo); zeroed once -- loads only
            # ever write the 56x56 interior of each 58x58 plane
            xts = []
            for s in range(2):
                row = []
                for i in range(NIMG):
                    xt = xppool.tile([P, 2, PADN], fp8, tag=f"xt{s}{i}")
                    nc.vector.memset(xt[:], 0.0)
                    row.append(xt)
                xts.append(row)

            def interior(xt, half):
                return xt[:, half, 0 : HP * WP].rearrange(
                    "p (h w) -> p h w", w=WP
                )[:, 1 : 1 + H, 1 : 1 + W]

            def emit_loads(dst):
                for i in range(NIMG):
                    xq = xqpool.tile([P, H, W], f32, tag="xq", name=f"xq{dst}_{i}")
                    nc.sync.dma_start(xq[:], x_d[i])
                    xt = xts[dst][i]
                    nc.vector.tensor_copy(interior(xt, 0), xq[:])
                    nc.vector.tensor_sub(interior(xt, 1), xq[:], interior(xt, 0))

            def emit_plane(img, c, xt, tail=False):
                ob = obpool.tile(
                    [P, NT, HT, W], f32, tag="ob", name=f"ob{img}_{c}"
                )
                pstride = 2 * PADN
                xh = xt[:].tensor
                for t in range(NT):
                    ps = pspool.tile(
                        [P, FREE8], f32, tag="ps", name=f"ps{img}_{c}_{t}"
                    )
                    # 9 DoubleRow pairs (whi[k]: xhi then xlo; dim1 hops the
                    # hi->lo section), 4 DoubleRow pairs (wlo[2m], wlo[2m+1]:
                    # both xhi; dim1 hops between tap bases), 1 plain single
                    # (wlo[8] * xhi)
                    for j in range(9):
                        rhs = AP(
                            xh, t * FREE8 + BASES[j],
                            [[pstride, P], [PADN, 2], [1, FREE8]],
                        )
                        nc.tensor.matmul(
                            ps[:], wt[:, j, :, c, :], rhs,
                            start=(j == 0), stop=False, perf_mode=DR,
                        )
                    for m in range(4):
                        rhs = AP(
                            xh, t * FREE8 + BASES[2 * m],
                            [[pstride, P],
                             [BASES[2 * m + 1] - BASES[2 * m], 2],
                             [1, FREE8]],
                        )
                        nc.tensor.matmul(
                            ps[:], wt[:, 9 + m, :, c, :], rhs,
                            start=False, stop=False, perf_mode=DR,
                        )
                    rhs = AP(
                        xh, t * FREE8 + BASES[8],
                        [[pstride, P], [1, FREE8]],
                    )
                    nc.tensor.matmul(
                        ps[:], wt[:, 13, 0, c, :], rhs, start=False, stop=True
                    )
                    ps3 = ps[:].rearrange("p (r q) -> p r q", q=WP)[:, :, 0:W]
                    nc.scalar.activation(
                        ob[:, t], ps3,
                        mybir.ActivationFunctionType.Identity,
                        bias=bt[:, c : c + 1], scale=1.0 / WSCALE,
                    )
                    if tail:
                        nc.scalar.dma_start(out_d[img, c, :, t, :], ob[:, t])
                    elif t == 2:
                        nc.scalar.dma_start(out_d[img, c, :, 0:3, :], ob[:, 0:3])
                if not tail:
                    nc.scalar.dma_start(out_d[img, c, :, 3:NT, :], ob[:, 3:NT])

            def emit_pass(src, dst=None, last=False):
                if dst is not None:
                    emit_loads(dst)
                for c in range(NCHUNK):
                    for img in range(NIMG):
                        emit_plane(
                            img, c, xts[src][img],
                            tail=last and c == NCHUNK - 1 and img == NIMG - 1,
                        )

            emit_loads(0)

            if repeat == 1:
                emit_pass(0, None, last=True)
            else:
                if unroll:
                    for _ in range(repeat // 2):
                        emit_pass(0, 1)
                        emit_pass(1, 0)
                else:
                    with tc.For_i(
                        0, repeat // 2, 1,
                        staggered_reset=True,
                        hint_engines=(
                            mybir.EngineType.PE,
                            mybir.EngineType.SP,
                            mybir.EngineType.Activation,
                            mybir.EngineType.DVE,
                        ),
                    ):
                        emit_pass(0, 1)
                        emit_pass(1, 0)
                if repeat % 2:
                    emit_pass(0, None, last=True)

    nc.compile()
    return nc


def _use_fp8():
    import os

    return os.environ.get("K_FP8", "0") == "1"


def kernel(x: np.ndarray, weight: np.ndarray, bias: np.ndarray) -> np.ndarray:
    from concourse.bass_utils import run_bass_kernel_spmd

    if "nc" not in _CACHE:
        _CACHE["nc"] = _build_fp8() if _use_fp8() else _build()
    nc = _CACHE["nc"]

    in_maps = [m for m in make_in_maps(x, weight, bias)]
    res = run_bass_kernel_spmd(nc, in_maps, list(range(NCORES)))
    out = np.concatenate(
        [r["out"].reshape(NIMG, C_OUT, H, W) for r in res.results], axis=0
    )
    return out


def make_in_maps(x, weight, bias):
    x = np.ascontiguousarray(x, dtype=np.float32)
    # w layout: [ci, kh*KW+kw, c, co_within_chunk]
    w_t = np.ascontiguousarray(
        weight.astype(np.float32)
        .transpose(1, 2, 3, 0)
        .reshape(P, KHW, NCHUNK, P)
    )
    b_t = np.ascontiguousarray(bias.astype(np.float32).reshape(NCHUNK, P).T)
    base = {"x": None, "w": w_t, "b": b_t}
    if _use_fp8():
        import ml_dtypes

        f8 = ml_dtypes.float8_e4m3
        wS = w_t * WSCALE
        w_hi = wS.astype(f8)
        w_lo = (wS - w_hi.astype(np.float32)).astype(f8)
        wp8 = np.zeros((P, NPAIR, 2, NCHUNK, P), f8)
        for j in range(KHW):           # pairs 0-8: whi[j] in both slots
            wp8[:, j, 0] = w_hi[:, j]
            wp8[:, j, 1] = w_hi[:, j]
        for m in range(4):             # pairs 9-12: (wlo[2m], wlo[2m+1])
            wp8[:, 9 + m, 0] = w_lo[:, 2 * m]
            wp8[:, 9 + m, 1] = w_lo[:, 2 * m + 1]
        wp8[:, 13, 0] = w_lo[:, 8]     # single
        base["w8"] = wp8
    return [
        {**base, "x": x[i * NIMG : (i + 1) * NIMG]} for i in range(NCORES)
    ]
